# revision 26
# baseline (speedup 1.0000x reference)
"""Trainium2 Bass kernel for nn_AttenBlock (sparse_attention).

Strategy: data-parallel over batch across 8 NeuronCores (4 batches/core).
Per core, a feature-major bf16 pipeline:
  LN(x) -> conv1d(last_x) -> softmax attention over 3-tap windows ->
  coarse/fine window mix -> SwiGLU -> down_proj -> token_proj.
All matmuls run on the PE array in bf16 (fp32 PSUM accumulation) with
moving free dim 512. Host pre-transposes activations/weights into the
SBUF layouts so no on-device transposes are needed.

The emission is software-pipelined: the front-end of batch b+1 is emitted
interleaved with the SwiGLU/token phase of batch b so the serial
LN/softmax/window chains (DVE/ACT latency) hide under dense PE work.

Token stream layout per batch: 1026 columns = [zero | 1021 tokens | 3 zero
gap/pad cols + zero edge]. Gap columns make the 3-tap conv/window halos
batch-local, and token_proj weights are zero-padded so gap tokens never
contribute to the output.
"""

import sys

sys.path.insert(0, "/opt/trn_rl_repo")

import numpy as np
import ml_dtypes

import concourse.bass as bass
import concourse.mybir as mybir
import concourse.tile as tile
from concourse import bacc
from concourse.bass_utils import run_bass_kernel_spmd

# problem shapes (hardcoded; harness provides matching inputs)
B, L, D = 32, 1021, 512
INTER, MOUT = 2048, 4096
NCORES = 8
BPC = B // NCORES        # batches per core
KT_D = D // 128          # 4  feature k-tiles
KT_2D = 2 * KT_D         # 8  feat k-tiles
NIT = INTER // 128       # 16 inter tiles
LP = 1026                # padded token columns per batch
CH = 512                 # token chunk
NCH = 2                  # chunks per batch
LTB = 8                  # 128-token l-tiles per batch
MT = MOUT // 128         # 32 output m-tiles

fp32 = mybir.dt.float32
CDT = mybir.dt.bfloat16
NPDT = ml_dtypes.bfloat16
AF = mybir.ActivationFunctionType

LAST_EXEC_NS = None
LAST_RESULTS = None
_NC_CACHE = None

import os as _os

DEBUG = bool(int(_os.environ.get("KERNEL_DEBUG", "0")))


def _emit(nc):
    x_t = nc.dram_tensor("x_t", [BPC, 128, KT_D, LP], CDT, kind="ExternalInput")
    lxin = nc.dram_tensor("lx_t", [BPC, 128, KT_D, LP], CDT, kind="ExternalInput")
    conv_wt = nc.dram_tensor("conv_wt", [128, KT_D, 3, D], CDT, kind="ExternalInput")
    atten_wt = nc.dram_tensor("atten_wt", [128, KT_D, 3], CDT, kind="ExternalInput")
    gate_wt = nc.dram_tensor("gate_wt", [128, NIT, KT_2D, 128], CDT, kind="ExternalInput")
    up_wt = nc.dram_tensor("up_wt", [128, NIT, KT_2D, 128], CDT, kind="ExternalInput")
    down_wt = nc.dram_tensor("down_wt", [128, NIT, D], CDT, kind="ExternalInput")
    tok_wt = nc.dram_tensor("tok_wt", [128, MT, LTB, 128], CDT, kind="ExternalInput")
    g_in = nc.dram_tensor("g_in", [128, KT_D], fp32, kind="ExternalInput")
    b_in = nc.dram_tensor("b_in", [128, KT_D], fp32, kind="ExternalInput")
    out_h = nc.dram_tensor("out", [BPC, MOUT, D], fp32, kind="ExternalOutput")
    if DEBUG:
        dbg_lxT = nc.dram_tensor("dbg_lxT", [BPC, 128, KT_D, LP], CDT, kind="ExternalOutput")
        dbg_xnT = nc.dram_tensor("dbg_xnT", [BPC, 128, KT_D, LP], CDT, kind="ExternalOutput")
        dbg_batt = nc.dram_tensor("dbg_batt", [BPC, 128, 3, NCH * CH], CDT, kind="ExternalOutput")
        dbg_feat = nc.dram_tensor("dbg_feat", [BPC, 128, KT_2D, NCH * CH], CDT, kind="ExternalOutput")
        dbg_y = nc.dram_tensor("dbg_y", [BPC, 128, LTB, D], CDT, kind="ExternalOutput")

    from contextlib import ExitStack

    with tile.TileContext(nc) as tc, ExitStack() as ctx:
        singles = ctx.enter_context(tc.tile_pool(name="singles", bufs=1))
        px = ctx.enter_context(tc.tile_pool(name="px", bufs=1))
        plx = ctx.enter_context(tc.tile_pool(name="plx", bufs=1))
        pfront = ctx.enter_context(tc.tile_pool(name="pfront", bufs=2))
        pbatt = ctx.enter_context(tc.tile_pool(name="pbatt", bufs=2))
        pfeat = ctx.enter_context(tc.tile_pool(name="pfeat", bufs=4))
        pstatf = ctx.enter_context(tc.tile_pool(name="pstatf", bufs=4))
        pstatc = ctx.enter_context(tc.tile_pool(name="pstatc", bufs=4))
        psq = ctx.enter_context(tc.tile_pool(name="psq", bufs=2))
        ptmpf = ctx.enter_context(tc.tile_pool(name="ptmpf", bufs=3))
        ptmpc = ctx.enter_context(tc.tile_pool(name="ptmpc", bufs=3))
        pgu = ctx.enter_context(tc.tile_pool(name="pgu", bufs=4))
        ph = ctx.enter_context(tc.tile_pool(name="ph", bufs=3))
        ptw = ctx.enter_context(tc.tile_pool(name="ptw", bufs=4))
        py_sb = ctx.enter_context(tc.tile_pool(name="py_sb", bufs=1))
        pout = ctx.enter_context(tc.tile_pool(name="pout", bufs=2))
        pgen = ctx.enter_context(tc.tile_pool(name="pgen", bufs=2, space="PSUM"))
        pfps = ctx.enter_context(tc.tile_pool(name="pfps", bufs=2, space="PSUM"))
        pyps = ctx.enter_context(tc.tile_pool(name="pyps", bufs=4, space="PSUM"))

        # ---- resident weights / constants ----
        conv_sb = singles.tile([128, KT_D, 3, D], CDT)
        nc.sync.dma_start(conv_sb[:], conv_wt[:])
        atten_sb = singles.tile([128, KT_D, 3], CDT)
        nc.sync.dma_start(atten_sb[:], atten_wt[:])
        down_sb = singles.tile([128, NIT, D], CDT)
        nc.sync.dma_start(down_sb[:], down_wt[:])
        g_sb = singles.tile([128, KT_D], fp32)
        nc.sync.dma_start(g_sb[:], g_in[:])
        b_sb = singles.tile([128, KT_D], fp32)
        nc.sync.dma_start(b_sb[:], b_in[:])
        ones_col = singles.tile([128, 1], CDT)
        nc.vector.memset(ones_col[:], 1.0)
        ones_row = singles.tile([1, 128], CDT)
        nc.vector.memset(ones_row[:], 1.0)
        eps_t = singles.tile([1, 1], fp32)
        nc.vector.memset(eps_t[:], 1e-5)

        state = {}

        def front(b):
            """Generator: emits the front-end of batch b in pieces."""
            st = state[b] = {}
            xt = px.tile([128, KT_D, LP], CDT, tag="xt", name=f"xt{b}")
            nc.sync.dma_start(xt[:], x_t[b])
            lxt = plx.tile([128, KT_D, LP], CDT, tag="lxt", name=f"lxt{b}")
            nc.sync.dma_start(lxt[:], lxin[b])
            lxT = pfront.tile([128, KT_D, LP], CDT, tag="lxT", name=f"lxT{b}")
            xnT = pfront.tile([128, KT_D, LP], CDT, tag="xnT", name=f"xnT{b}")
            batt_cs = [pbatt.tile([128, 3, CH], CDT, tag="batt", name=f"batt{b}_{c}")
                       for c in range(NCH)]
            feat_cs = [pfeat.tile([128, KT_2D, CH], CDT, tag="feat", name=f"feat{b}_{c}")
                       for c in range(NCH)]
            st["lxT"], st["xnT"] = lxT, xnT
            st["batt_cs"], st["feat_cs"] = batt_cs, feat_cs
            yield

            for c in range(NCH):
                sl = slice(1 + c * CH, 1 + c * CH + CH)
                # ---- layernorm stats (feature-major, via ones-matmul) ----
                s1 = pfps.tile([128, CH], fp32, tag="fps", name="s1")
                for kt in range(KT_D):
                    nc.tensor.matmul(s1[:1], ones_col[:], xt[:, kt, sl],
                                     start=(kt == 0), stop=(kt == KT_D - 1))
                s2 = pfps.tile([128, CH], fp32, tag="fps", name="s2")
                for kt in range(KT_D):
                    sq = psq.tile([128, CH], CDT, tag="sq", name="sq")
                    nc.scalar.square(sq[:], xt[:, kt, sl])
                    nc.tensor.matmul(s2[:1], ones_col[:], sq[:],
                                     start=(kt == 0), stop=(kt == KT_D - 1))
                yield
                mean_f = pstatf.tile([1, CH], fp32, tag="stf", name="mean_f")
                nc.vector.tensor_scalar_mul(mean_f[:], s1[:1], 1.0 / D)
                mean_c = pstatc.tile([1, CH], CDT, tag="stc", name="mean_c")
                nc.gpsimd.tensor_copy(mean_c[:], mean_f[:])
                msq = pstatf.tile([1, CH], fp32, tag="stf", name="msq")
                nc.vector.tensor_mul(msq[:], mean_f[:], mean_f[:])
                var = pstatf.tile([1, CH], fp32, tag="stf", name="var")
                nc.vector.tensor_scalar_mul(var[:], s2[:1], 1.0 / D)
                nc.vector.tensor_sub(var[:], var[:], msq[:])
                nc.scalar.activation(var[:], var[:], AF.Sqrt, bias=eps_t[:])
                rstd_c = pstatc.tile([1, CH], CDT, tag="stc", name="rstd_c")
                with nc.allow_low_precision(reason="bf16 rstd, bf16 pipeline"):
                    nc.vector.reciprocal(rstd_c[:], var[:])
                # broadcast mean/rstd across partitions via K=1 matmul
                mb_ps = pfps.tile([128, CH], fp32, tag="fps", name="mb_ps")
                nc.tensor.matmul(mb_ps[:], ones_row[:], mean_c[:], start=True, stop=True)
                rb_ps = pfps.tile([128, CH], fp32, tag="fps", name="rb_ps")
                nc.tensor.matmul(rb_ps[:], ones_row[:], rstd_c[:], start=True, stop=True)
                yield
                for kt in range(KT_D):
                    t = ptmpf.tile([128, CH], fp32, tag="lnt", name="lnt")
                    nc.vector.tensor_sub(t[:], xt[:, kt, sl], mb_ps[:])
                    nc.vector.tensor_mul(t[:], t[:], rb_ps[:])
                    nc.scalar.activation(xnT[:, kt, sl], t[:], AF.Identity,
                                         bias=b_sb[:, kt:kt + 1],
                                         scale=g_sb[:, kt:kt + 1])
                yield

                # ---- conv1d (3-tap) -> lxT (feature-major) ----
                for do in range(KT_D):
                    cps = pfps.tile([128, CH], fp32, tag="fps", name="cps")
                    idx = 0
                    for tap in range(3):
                        for kin in range(KT_D):
                            nc.tensor.matmul(
                                cps[:],
                                conv_sb[:, kin, tap, do * 128:(do + 1) * 128],
                                lxt[:, kin, c * CH + tap:c * CH + tap + CH],
                                start=(idx == 0), stop=(idx == 11))
                            idx += 1
                    nc.vector.tensor_copy(lxT[:, do, sl], cps[:])
                    yield

            # zero the gap/edge columns (conv output at gaps is garbage;
            # windows at batch edges must read zeros)
            nc.gpsimd.memset(lxT[:, :, 0:1], 0.0)
            nc.gpsimd.memset(lxT[:, :, 1022:1026], 0.0)
            nc.gpsimd.memset(xnT[:, :, 0:1], 0.0)
            nc.gpsimd.memset(xnT[:, :, 1022:1026], 0.0)
            yield

            for c in range(NCH):
                sl = slice(1 + c * CH, 1 + c * CH + CH)
                # ---- attention logits + softmax over 3 taps ----
                lg_ps = pfps.tile([128, CH], fp32, tag="fps", name="lg_ps")
                for kin in range(KT_D):
                    nc.tensor.matmul(lg_ps[:3], atten_sb[:, kin, :],
                                     lxT[:, kin, sl],
                                     start=(kin == 0), stop=(kin == KT_D - 1))
                exp_sb = pstatc.tile([3, CH], CDT, tag="exp3", name="exp_sb")
                nc.scalar.activation(exp_sb[:], lg_ps[:3], AF.Exp)
                expk = [pstatc.tile([1, CH], CDT, tag="expk", name=f"expk{k}")
                        for k in range(3)]
                for k in range(3):
                    nc.sync.dma_start(expk[k][:], exp_sb[k:k + 1, :])
                yield
                den = pstatf.tile([1, CH], fp32, tag="stf", name="den")
                nc.vector.tensor_add(den[:], expk[0][:], expk[1][:])
                nc.vector.tensor_add(den[:], den[:], expk[2][:])
                rec_c = pstatc.tile([1, CH], CDT, tag="stc", name="rec_c")
                with nc.allow_low_precision(reason="bf16 softmax denom"):
                    nc.vector.reciprocal(rec_c[:], den[:])
                yield
                brec_ps = pfps.tile([128, CH], fp32, tag="fps", name="brec_ps")
                nc.tensor.matmul(brec_ps[:], ones_row[:], rec_c[:], start=True, stop=True)
                brec_sb = ptmpc.tile([128, CH], CDT, tag="brec", name="brec_sb")
                nc.scalar.copy(brec_sb[:], brec_ps[:])
                bt = batt_cs[c]
                for k in range(3):
                    bex_ps = pfps.tile([128, CH], fp32, tag="fps", name="bex_ps")
                    nc.tensor.matmul(bex_ps[:], ones_row[:], expk[k][:], start=True, stop=True)
                    nc.vector.tensor_mul(bt[:, k, :], bex_ps[:], brec_sb[:])
                yield

                # ---- windowed coarse/fine mix -> feat_cs[c] ----
                ft = feat_cs[c]
                for do in range(KT_D):
                    t1 = ptmpc.tile([128, CH], CDT, tag="w1", name="w1")
                    t2 = ptmpc.tile([128, CH], CDT, tag="w2", name="w2")
                    nc.vector.tensor_mul(t1[:], bt[:, 0, :], lxT[:, do, c * CH:c * CH + CH])
                    nc.vector.tensor_mul(t2[:], bt[:, 1, :], lxT[:, do, c * CH + 1:c * CH + 1 + CH])
                    nc.vector.tensor_add(t1[:], t1[:], t2[:])
                    nc.vector.tensor_mul(t2[:], bt[:, 2, :], lxT[:, do, c * CH + 2:c * CH + 2 + CH])
                    nc.vector.tensor_add(ft[:, do, :], t1[:], t2[:])
                    f1 = ptmpc.tile([128, CH], CDT, tag="f1", name="f1")
                    f2 = ptmpc.tile([128, CH], CDT, tag="f2", name="f2")
                    nc.vector.tensor_mul(f1[:], bt[:, 0, :], xnT[:, do, c * CH:c * CH + CH])
                    nc.vector.tensor_mul(f2[:], bt[:, 1, :], xnT[:, do, c * CH + 1:c * CH + 1 + CH])
                    nc.vector.tensor_add(f1[:], f1[:], f2[:])
                    nc.vector.tensor_mul(f2[:], bt[:, 2, :], xnT[:, do, c * CH + 2:c * CH + 2 + CH])
                    nc.vector.tensor_add(ft[:, KT_D + do, :], f1[:], f2[:])
                    yield

        def heavy(b, fg):
            """Emit SwiGLU+down+token of batch b, advancing front generator
            fg (batch b+1) at interleave points."""

            def tick():
                if fg is not None:
                    try:
                        next(fg)
                    except StopIteration:
                        pass

            st = state[b]
            feat_cs = st["feat_cs"]
            y_b = py_sb.tile([128, LTB, D], CDT, tag="y_b", name=f"y_b{b}")

            for c in range(NCH):
                ft = feat_cs[c]
                y_ps = [pyps.tile([128, D], fp32, tag="y", name=f"y_ps{lt}")
                        for lt in range(4)]
                for i in range(NIT):
                    gw = pgu.tile([128, KT_2D, 128], CDT, tag="gw", name="gw")
                    nc.sync.dma_start(gw[:], gate_wt[:, i])
                    uw = pgu.tile([128, KT_2D, 128], CDT, tag="uw", name="uw")
                    nc.sync.dma_start(uw[:], up_wt[:, i])
                    g_ps = pgen.tile([128, CH], fp32, tag="ps", name="g_ps")
                    for kt in range(KT_2D):
                        nc.tensor.matmul(g_ps[:], gw[:, kt, :], ft[:, kt, :],
                                         start=(kt == 0), stop=(kt == KT_2D - 1))
                    u_ps = pgen.tile([128, CH], fp32, tag="ps", name="u_ps")
                    for kt in range(KT_2D):
                        nc.tensor.matmul(u_ps[:], uw[:, kt, :], ft[:, kt, :],
                                         start=(kt == 0), stop=(kt == KT_2D - 1))
                    sg = ph.tile([128, CH], fp32, tag="sg", name="sg")
                    nc.scalar.activation(sg[:], g_ps[:], AF.Silu)
                    h = ph.tile([128, CH], CDT, tag="h", name="h")
                    nc.vector.tensor_mul(h[:], sg[:], u_ps[:])
                    for lt in range(4):
                        nc.tensor.matmul(y_ps[lt][:], h[:, lt * 128:(lt + 1) * 128],
                                         down_sb[:, i, :],
                                         start=(i == 0), stop=(i == NIT - 1))
                    if i % 2 == 1:
                        tick()
                for lt in range(4):
                    if lt % 2 == 0:
                        nc.scalar.copy(y_b[:, 4 * c + lt, :], y_ps[lt][:])
                    else:
                        nc.vector.tensor_copy(y_b[:, 4 * c + lt, :], y_ps[lt][:])
                tick()

            if DEBUG:
                nc.sync.dma_start(dbg_lxT[b], st["lxT"][:])
                nc.sync.dma_start(dbg_xnT[b], st["xnT"][:])
                for c in range(NCH):
                    nc.sync.dma_start(dbg_batt[b, :, :, c * CH:(c + 1) * CH],
                                      st["batt_cs"][c][:])
                    nc.sync.dma_start(dbg_feat[b, :, :, c * CH:(c + 1) * CH],
                                      st["feat_cs"][c][:])
                nc.sync.dma_start(dbg_y[b], y_b[:])

            # ---- token_proj: out[m, d] = sum_l tok_w[l, m] * y[l, d] ----
            for m in range(MT):
                tw = ptw.tile([128, LTB, 128], CDT, tag="tw", name="tw")
                nc.sync.dma_start(tw[:], tok_wt[:, m])
                t_ps = pyps.tile([128, D], fp32, tag="y", name="t_ps")
                for lt in range(LTB):
                    nc.tensor.matmul(t_ps[:], tw[:, lt, :], y_b[:, lt, :],
                                     start=(lt == 0), stop=(lt == LTB - 1))
                o_sb = pout.tile([128, D], fp32, tag="o_sb", name="o_sb")
                if m % 2 == 0:
                    nc.scalar.copy(o_sb[:], t_ps[:])
                else:
                    nc.vector.tensor_copy(o_sb[:], t_ps[:])
                nc.sync.dma_start(out_h[b, m * 128:(m + 1) * 128, :], o_sb[:])
                if m % 2 == 1:
                    tick()
            # drain any remaining front pieces
            if fg is not None:
                for _ in fg:
                    pass

        # software pipeline: front(0) fully, then heavy(b) ∥ front(b+1)
        for _ in front(0):
            pass
        for b in range(BPC):
            fg = front(b + 1) if b + 1 < BPC else None
            heavy(b, fg)

    return nc


def _get_nc():
    global _NC_CACHE
    if _NC_CACHE is None:
        nc = bacc.Bacc("TRN2", target_bir_lowering=False, debug=False,
                       num_devices=NCORES)
        _emit(nc)
        nc.compile()
        nc.finalize()
        _NC_CACHE = nc
    return _NC_CACHE


def _prep_host(inputs):
    x = np.asarray(inputs["x"], np.float32)
    last_x = np.asarray(inputs["last_x"], np.float32)
    ln_g = np.asarray(inputs["ln_g"], np.float32)
    ln_b = np.asarray(inputs["ln_b"], np.float32)
    conv_w = np.asarray(inputs["conv_w"], np.float32)
    atten_w = np.asarray(inputs["atten_w"], np.float32)
    gate_w = np.asarray(inputs["gate_w"], np.float32)
    up_w = np.asarray(inputs["up_w"], np.float32)
    down_w = np.asarray(inputs["down_w"], np.float32)
    token_w = np.asarray(inputs["token_w"], np.float32)

    conv_a = np.ascontiguousarray(
        conv_w.transpose(1, 2, 0).reshape(KT_D, 128, 3, D).transpose(1, 0, 2, 3)
    ).astype(NPDT)
    atten_a = np.ascontiguousarray(
        atten_w.T.reshape(KT_D, 128, 3).transpose(1, 0, 2)).astype(NPDT)
    gate_a = np.ascontiguousarray(
        gate_w.T.reshape(KT_2D, 128, NIT, 128).transpose(1, 2, 0, 3)).astype(NPDT)
    up_a = np.ascontiguousarray(
        up_w.T.reshape(KT_2D, 128, NIT, 128).transpose(1, 2, 0, 3)).astype(NPDT)
    down_a = np.ascontiguousarray(
        down_w.T.reshape(NIT, 128, D).transpose(1, 0, 2)).astype(NPDT)
    twT = np.zeros((LTB * 128, MOUT), np.float32)
    twT[:L] = token_w.T
    tok_a = np.ascontiguousarray(
        twT.reshape(LTB, 128, MT, 128).transpose(1, 2, 0, 3)).astype(NPDT)
    g_a = np.ascontiguousarray(ln_g.reshape(KT_D, 128).T).astype(np.float32)
    b_a = np.ascontiguousarray(ln_b.reshape(KT_D, 128).T).astype(np.float32)

    def tr(t):  # [BPC, L, D] -> [BPC, 128, KT_D, LP] padded feature-major
        buf = np.zeros((BPC, D, LP), np.float32)
        buf[:, :, 1:1 + L] = t.transpose(0, 2, 1)
        return np.ascontiguousarray(
            buf.reshape(BPC, KT_D, 128, LP).transpose(0, 2, 1, 3)).astype(NPDT)

    in_maps = []
    for c in range(NCORES):
        s = slice(c * BPC, (c + 1) * BPC)
        in_maps.append({
            "x_t": tr(x[s]), "lx_t": tr(last_x[s]),
            "conv_wt": conv_a, "atten_wt": atten_a,
            "gate_wt": gate_a, "up_wt": up_a, "down_wt": down_a,
            "tok_wt": tok_a, "g_in": g_a, "b_in": b_a,
        })
    return in_maps


def kernel(**inputs):
    global LAST_EXEC_NS, LAST_RESULTS
    import os
    in_maps = _prep_host(inputs)
    nc = _get_nc()
    trace = bool(int(os.environ.get("KERNEL_TRACE", "0")))
    res = run_bass_kernel_spmd(nc, in_maps, core_ids=list(range(NCORES)),
                               trace=trace)
    LAST_EXEC_NS = res.exec_time_ns
    LAST_RESULTS = res.results
    return np.concatenate([r["out"] for r in res.results], axis=0)
